# revision 1
# baseline (speedup 1.0000x reference)
"""Trainium2 Bass kernel for nn_DenoisingNet (GNN message passing).

Strategy
--------
The per-edge attention MLP decomposes into per-node scalars:
    log_alpha[e] = a1[row[e]] + a2[col[e]] + b_att
    a1[n] = relu(x[n] @ W_nb  + b_nb)  @ W_att[:128]
    a2[n] = relu(x[n] @ W_self + b_self) @ W_att[128:]
so the MLP runs over N=50k nodes instead of E=800k edges (16x less).

Sharding: edges are bucketed by row-range (core c owns rows
[c*6250, (c+1)*6250)), row-sorted, whole nodes packed per SBUF
partition.  Per-node rowsums are then core-local (no all-reduce); two
small AllGathers share the a2 / d_inv node tables across cores.

Indexed access uses the MoE `dma_gather` ucode (the only fast indexed
DMA on TRN2): int16 indices wrapped-16, 256-byte table rows.  Node
tables are stored 64-wide ([n, 64] f32, all columns equal; the a-matmul
produces this for free with a broadcast rhs).  The 50176-row global
tables exceed int16 range, so col-side gathers run twice (rows <32768
and >=32768) into two buffers and a predicated select merges them.
Rowsum falls out of an inclusive prefix scan (native tensor_tensor_scan)
plus a small boundary gather ([128,1]-offset indirect DMA per node
column - the only HW-correct indirect form, see hw_probe.py).
"""

import functools

import numpy as np

import concourse.bacc as bacc
import concourse.bass as bass
import concourse.tile as tile
from concourse import mybir
from concourse.bass import IndirectOffsetOnAxis
from concourse.bass_utils import run_bass_kernel_spmd
from concourse.masks import make_identity
from concourse.tile import add_dep_helper

# ---- problem constants (hardcoded per contest contract) ----
N = 50000
E = 800000
D = 256
H = 128
CORES = 8
NPC = N // CORES          # 6250 nodes per core
NT = 49                   # node tiles per core (padded)
LN = 128 * NT             # 6272 padded local nodes
FULLN = CORES * LN        # 50176 padded global nodes
KMAX = 80                 # max packed nodes per edge-partition
F = 864                   # edge slots per partition
SLOTS = 128 * F           # 110592
GW = F + 1                # gcum row width (col 0 = sentinel zero)
GLEN = 128 * GW           # 110720
ES = 64                   # wide-table row: 64 f32 = 256 B (dma_gather min)
CH = 54                   # gather chunk: 54 slots/partition
NCHUNK = F // CH          # 16 chunks
NI = 128 * CH             # 6912 indices per chunk
IW = NI // 16             # 432 wrapped-idx columns per chunk
HI0 = 32768               # int16 split point for the global tables

GAMMA = -0.5
ZETA = 1.1
DEBUG_VAR = 1e-07
DEBUG = False

f32 = mybir.dt.float32
i32 = mybir.dt.int32
i16 = mybir.dt.int16
AF = mybir.ActivationFunctionType
OP = mybir.AluOpType


# ======================================================================
# host-side sharding
# ======================================================================

def _tpos(l):
    """Flat position of local node id l in the device node tables.

    The a-matmul for node tile t emits node t*128+m on partition m, so
    the [128, NT] tables flatten as (m, t) -> node t*128+m."""
    return (l % 128) * NT + (l // 128)


def _wrap16(arr):
    """Per-edge index grid [128, F] -> wrapped int16 chunks [128, NCHUNK*IW].

    dma_gather consumes indices in logical order i -> (partition i%128,
    group i//128), stored wrapped-16: index i at [i%16, i//16],
    replicated across the 8 Q7 cores (16-partition groups)."""
    out = np.empty((128, NCHUNK * IW), np.int16)
    for c in range(NCHUNK):
        sub = arr[:, c * CH:(c + 1) * CH]        # [128, CH]
        L = sub.T.ravel()                        # L[i] = sub[i%128, i//128]
        w = L.reshape(IW, 16).T.astype(np.int16)  # [16, IW]
        out[:, c * IW:(c + 1) * IW] = np.tile(w, (8, 1))
    return out


def _pack_core(c, row, col, values, noise):
    """Row-bucketed, row-sorted, partition-packed layout for one core."""
    gsel = np.where(row // NPC == c)[0]
    lr = row[gsel] - c * NPC
    order = np.argsort(lr, kind="stable")
    eidx = gsel[order]
    lr = lr[order]
    ne = len(eidx)
    deg = np.bincount(lr, minlength=NPC)
    assert deg.max() <= F, f"node degree {deg.max()} > {F}"

    part_of_node = np.zeros(NPC, np.int32)
    node_slot_start = np.zeros(NPC, np.int64)
    p = 0
    slots_p = 0
    k_p = 0
    placed = 0
    target = -(-ne // 128)
    for l in range(NPC):
        d = int(deg[l])
        if p < 127 and (slots_p + d > target or k_p >= KMAX):
            p += 1
            slots_p = 0
            k_p = 0
            target = -(-(ne - placed) // (128 - p))
        assert slots_p + d <= F, f"partition overflow {slots_p}+{d}"
        part_of_node[l] = p
        node_slot_start[l] = slots_p
        slots_p += d
        k_p += 1
        placed += d

    vals_s = np.zeros((128, F), np.float32)
    noise_s = np.full((128, F), 0.5, np.float32)
    colg = np.zeros((128, F), np.int64)   # global table index per slot
    rowt = np.zeros((128, F), np.int64)   # local table pos of row node
    perm = np.full((128, F), -1, np.int64)

    node_first = np.searchsorted(lr, np.arange(NPC), side="left")
    pos_in_node = np.arange(ne) - node_first[lr]
    pslot = part_of_node[lr]
    jslot = node_slot_start[lr] + pos_in_node
    vals_s[pslot, jslot] = values[eidx]
    noise_s[pslot, jslot] = noise[eidx, 0]
    gcol = col[eidx]
    colg[pslot, jslot] = (gcol // NPC) * LN + _tpos(gcol % NPC)
    rowt[pslot, jslot] = _tpos(lr)
    perm[pslot, jslot] = eidx

    # boundary positions for rowsum (into gcum [128, GW] flattened)
    P1 = np.zeros(LN, np.int32)
    P0 = np.zeros(LN, np.int32)
    for l in range(NPC):
        pp = part_of_node[l]
        j0 = int(node_slot_start[l])
        P1[l] = pp * GW + j0 + int(deg[l])
        P0[l] = pp * GW + j0
    p1p0 = np.concatenate(
        [P1.reshape(NT, 128).T, P0.reshape(NT, 128).T], axis=1
    ).astype(np.int32)

    return dict(
        vals_s=vals_s, noise_s=noise_s, perm=perm, p1p0=p1p0,
        rowidx=rowt.astype(np.int32), colidx=colg.astype(np.int32),
    )


def make_in_maps(inputs):
    """Full inputs -> per-core input maps + unshard metadata."""
    x = np.ascontiguousarray(np.asarray(inputs["x"], np.float32))
    row = np.asarray(inputs["row"])
    col = np.asarray(inputs["col"])
    values = np.asarray(inputs["values"], np.float32)
    noise = np.asarray(inputs["noise"], np.float32)
    batt = np.full(128, np.asarray(inputs["b_att"], np.float32).reshape(-1)[0],
                   np.float32)

    in_maps = []
    perms = []
    for c in range(CORES):
        meta = _pack_core(c, row, col, values, noise)
        xs = np.zeros((LN, D), np.float32)
        xs[:NPC] = x[c * NPC:(c + 1) * NPC]
        in_maps.append({
            "x_shard": xs,
            "w_nb": np.asarray(inputs["W_nb"], np.float32),
            "w_self": np.asarray(inputs["W_self"], np.float32),
            "b_nb": np.asarray(inputs["b_nb"], np.float32),
            "b_self": np.asarray(inputs["b_self"], np.float32),
            "watt": np.asarray(inputs["W_att"], np.float32).reshape(-1),
            "batt": batt,
            "vals_s": meta["vals_s"],
            "noise_s": meta["noise_s"],
            "rowidx": meta["rowidx"],
            "colidx": meta["colidx"],
            "p1p0": meta["p1p0"],
        })
        perms.append(meta["perm"])
    return in_maps, perms


def unshard(results, perms):
    out = np.zeros(E, np.float32)
    for c in range(CORES):
        o = np.asarray(results[c]["out_s"])
        m = perms[c] >= 0
        out[perms[c][m]] = o[m]
    return out


# ======================================================================
# device program
# ======================================================================

def _build_body(tc):
    nc = tc.nc

    def din(name, shape, dtype=f32):
        return nc.dram_tensor(name, shape, dtype, kind="ExternalInput").ap()

    x_d = din("x_shard", [LN, D])
    wnb_d = din("w_nb", [D, H])
    wself_d = din("w_self", [D, H])
    bnb_d = din("b_nb", [H])
    bself_d = din("b_self", [H])
    watt_d = din("watt", [2 * H])
    batt_d = din("batt", [128])
    vals_d = din("vals_s", [128, F])
    noise_d = din("noise_s", [128, F])
    rowidx_d = din("rowidx", [128, F], i32)
    colidx_d = din("colidx", [128, F], i32)
    p1p0_d = din("p1p0", [128, 2 * NT], i32)

    out_d = nc.dram_tensor("out_s", [128, F], f32, kind="ExternalOutput").ap()

    a1loc = nc.dram_tensor("a1loc", [LN], f32).ap()
    a2loc = nc.dram_tensor("a2loc", [LN], f32).ap()
    dinvloc = nc.dram_tensor("dinvloc", [LN], f32).ap()
    a2full = nc.dram_tensor("a2full", [FULLN], f32, addr_space="Shared").ap()
    dinvfull = nc.dram_tensor("dinvfull", [FULLN], f32,
                              addr_space="Shared").ap()
    gcum = nc.dram_tensor("gcum", [GLEN], f32).ap()

    groups = [list(range(CORES))]

    from contextlib import ExitStack
    ctx = _build_body.ctx
    const = ctx.enter_context(tc.tile_pool(name="const", bufs=1))
    meta = ctx.enter_context(tc.tile_pool(name="meta", bufs=1))
    p1 = ExitStack()
    xload = p1.enter_context(tc.tile_pool(name="xload", bufs=4))
    xtp = p1.enter_context(tc.tile_pool(name="xt", bufs=1))
    hrp = p1.enter_context(tc.tile_pool(name="hr", bufs=1))
    tp_ps = p1.enter_context(tc.tile_pool(name="tp_ps", bufs=2, space="PSUM"))
    mm_ps = p1.enter_context(tc.tile_pool(name="mm_ps", bufs=2, space="PSUM"))
    a_ps = p1.enter_context(tc.tile_pool(name="a_ps", bufs=1, space="PSUM"))

    # ---------------- constants ----------------
    ident = const.tile([128, 128], f32)
    make_identity(nc, ident[:])
    w_sb = {}
    for nm, dram in (("nb", wnb_d), ("self", wself_d)):
        for k in range(2):
            t = const.tile([128, H], f32, tag=f"w_{nm}{k}", name=f"w_{nm}{k}")
            nc.sync.dma_start(out=t[:], in_=dram[k * 128:(k + 1) * 128, :])
            w_sb[(nm, k)] = t
    b_sb = {}
    for nm, dram in (("nb", bnb_d), ("self", bself_d)):
        t = const.tile([128, 1], f32, tag=f"b_{nm}", name=f"b_{nm}")
        nc.sync.dma_start(out=t[:], in_=dram[:, None])
        b_sb[nm] = t
    wv_sb = {}
    for nm, sl in (("nb", slice(0, 128)), ("self", slice(128, 256))):
        t = const.tile([128, 1], f32, tag=f"wv_{nm}", name=f"wv_{nm}")
        nc.sync.dma_start(out=t[:], in_=watt_d[sl, None])
        wv_sb[nm] = t
    batt_sb = const.tile([128, 1], f32)
    nc.sync.dma_start(out=batt_sb[:], in_=batt_d[:, None])
    def constf(val, nm):
        t = const.tile([128, 1], f32, tag=f"c_{nm}", name=f"c_{nm}")
        nc.vector.memset(t[:], val)
        return t

    cb_dv = constf(DEBUG_VAR, "dv")
    cb_1mdv = constf(1.0 - DEBUG_VAR, "odv")
    cb_gamma = constf(GAMMA, "gm")

    # ---------------- phase 1: transpose x, MLP, a1/a2 tables ----------
    xt = [xtp.tile([128, LN], f32, tag=f"xt{k}", name=f"xt{k}")
          for k in range(2)]
    for t in range(NT):
        xtile = xload.tile([128, D], f32)
        nc.sync.dma_start(out=xtile[:], in_=x_d[t * 128:(t + 1) * 128, :])
        for k in range(2):
            ps = tp_ps.tile([128, 128], f32)
            nc.tensor.transpose(
                out=ps[:], in_=xtile[:, k * 128:(k + 1) * 128],
                identity=ident[:]
            )
            nc.vector.tensor_copy(out=xt[k][:, t * 128:(t + 1) * 128],
                                  in_=ps[:])

    STRIP = 512
    strips = [(s, min(s + STRIP, LN)) for s in range(0, LN, STRIP)]
    a1w_stores = []
    a2_store = None
    # "self" half first: a2 feeds the AllGather on the critical path
    for nm in ("self", "nb"):
        hr = hrp.tile([128, LN], f32, tag="hr", name=f"hr_{nm}")
        for s0, s1 in strips:
            ps = mm_ps.tile([128, STRIP], f32)
            for k in range(2):
                nc.tensor.matmul(
                    out=ps[:, :s1 - s0],
                    lhsT=w_sb[(nm, k)][:],
                    rhs=xt[k][:, s0:s1],
                    start=(k == 0),
                    stop=(k == 1),
                )
            nc.scalar.activation(
                out=hr[:, s0:s1], in_=ps[:, :s1 - s0], func=AF.Relu,
                bias=b_sb[nm][:, 0:1],
            )
        aps = a_ps.tile([128, NT], f32, tag="aps", name=f"aps_{nm}", bufs=2)
        for t in range(NT):
            nc.tensor.matmul(
                out=aps[:, t:t + 1],
                lhsT=hr[:, t * 128:(t + 1) * 128],
                rhs=wv_sb[nm][:],
                start=True, stop=True,
            )
        a_sb = meta.tile([128, NT], f32, tag="a_sb", name=f"a_sb_{nm}")
        nc.vector.tensor_copy(out=a_sb[:], in_=aps[:])
        dst = a2loc if nm == "self" else a1loc
        st = nc.sync.dma_start(
            out=dst.rearrange("(p k) -> p k", p=128), in_=a_sb[:]
        )
        if nm == "self":
            a2_store = st
        else:
            a1w_stores.append(st)

    p1.close()
    edge = ctx.enter_context(tc.tile_pool(name="edge", bufs=1))

    # ---------------- AllGather a2, widen to [FULLN, ES] ----------------
    cc_a2 = nc.gpsimd.collective_compute(
        "AllGather", OP.bypass, replica_groups=groups,
        ins=[a2loc], outs=[a2full],
    )
    add_dep_helper(cc_a2.ins, a2_store.ins)

    # ---------------- edge inputs ----------------
    vals = edge.tile([128, F], f32)
    nc.sync.dma_start(out=vals[:], in_=vals_d[:])
    noise = edge.tile([128, F], f32)
    nc.sync.dma_start(out=noise[:], in_=noise_d[:])
    rowidx = edge.tile([128, F], i32)
    nc.sync.dma_start(out=rowidx[:], in_=rowidx_d[:])
    colidx = edge.tile([128, F], i32)
    nc.sync.dma_start(out=colidx[:], in_=colidx_d[:])
    p1p0 = meta.tile([128, 2 * NT], i32)
    nc.sync.dma_start(out=p1p0[:], in_=p1p0_d[:])

    # noise logit
    lnu = edge.tile([128, F], f32)
    nc.scalar.activation(out=lnu[:], in_=noise[:], func=AF.Ln,
                         bias=cb_dv[:, 0:1], scale=1.0)
    ln1mu = edge.tile([128, F], f32)
    nc.scalar.activation(out=ln1mu[:], in_=noise[:], func=AF.Ln,
                         bias=cb_1mdv[:, 0:1], scale=-1.0)
    nl = edge.tile([128, F], f32)
    nc.vector.tensor_sub(nl[:], lnu[:], ln1mu[:])

    def gather_cols(dst, table, idx_tile, deps, tag):
        """dst [128, F] <- table[idx] via per-column [128,1] indirects."""
        for c in range(F):
            g = nc.gpsimd.indirect_dma_start(
                out=dst[:, c:c + 1], out_offset=None, in_=table[:, None],
                in_offset=IndirectOffsetOnAxis(ap=idx_tile[:, c:c + 1],
                                               axis=0),
            )
            for d in deps:
                add_dep_helper(g.ins, d.ins)

    # a1 row expansion + a2 col gather
    a1exp = edge.tile([128, F], f32)
    gather_cols(a1exp, a1loc, rowidx, a1w_stores, "a1r")
    a2exp = edge.tile([128, F], f32)
    gather_cols(a2exp, a2full, colidx, [cc_a2], "a2c")

    # ---------------- mask & masked values (in-place chain) -----------
    nc.vector.tensor_add(nl[:], nl[:], a1exp[:])
    nc.vector.tensor_add(nl[:], nl[:], a2exp[:])
    gate = edge.tile([128, F], f32)
    nc.scalar.activation(out=gate[:], in_=nl[:], func=AF.Sigmoid,
                         bias=batt_sb[:, 0:1])
    nc.scalar.activation(out=gate[:], in_=gate[:], func=AF.Relu,
                         bias=cb_gamma[:, 0:1], scale=ZETA - GAMMA)
    nc.vector.tensor_scalar_min(gate[:], gate[:], 1.0)
    mv = edge.tile([128, F], f32)
    nc.vector.tensor_mul(mv[:], vals[:], gate[:])

    # ---------------- rowsum via scan + boundary gather ----------------
    gxt = edge.tile([128, GW], f32)
    nc.vector.memset(gxt[:, 0:1], 0.0)
    nc.vector.tensor_tensor_scan(
        out=gxt[:, 1:], data0=mv[:], data1=mv[:], initial=0.0,
        op0=OP.add, op1=OP.bypass,
    )
    wgc = nc.sync.dma_start(
        out=gcum.rearrange("(p w) -> p w", p=128), in_=gxt[:]
    )
    bb = meta.tile([128, 2 * NT], f32)
    for k in range(2 * NT):
        gb = nc.gpsimd.indirect_dma_start(
            out=bb[:, k:k + 1], out_offset=None, in_=gcum[:, None],
            in_offset=IndirectOffsetOnAxis(ap=p1p0[:, k:k + 1], axis=0),
        )
        add_dep_helper(gb.ins, wgc.ins)
    rowsum = meta.tile([128, NT], f32)
    nc.vector.tensor_sub(rowsum[:], bb[:, :NT], bb[:, NT:])
    rsp = meta.tile([128, NT], f32)
    nc.vector.tensor_scalar_add(rsp[:], rowsum[:], 1e-10)
    rcp = meta.tile([128, NT], f32)
    nc.vector.reciprocal(rcp[:], rsp[:])
    dinv = meta.tile([128, NT], f32)
    nc.scalar.activation(out=dinv[:], in_=rcp[:], func=AF.Sqrt)
    wdl = nc.sync.dma_start(
        out=dinvloc.rearrange("(p k) -> p k", p=128), in_=dinv[:]
    )
    # ---------------- AllGather d_inv + widen + final gathers -----------
    cc_di = nc.gpsimd.collective_compute(
        "AllGather", OP.bypass, replica_groups=groups,
        ins=[dinvloc], outs=[dinvfull],
    )
    add_dep_helper(cc_di.ins, wdl.ins)

    drow = edge.tile([128, F], f32)
    gather_cols(drow, dinvloc, rowidx, [wdl], "dir")
    nc.vector.tensor_mul(drow[:], mv[:], drow[:])

    dcol = edge.tile([128, F], f32)
    gather_cols(dcol, dinvfull, colidx, [cc_di], "dic")

    nc.vector.tensor_mul(dcol[:], drow[:], dcol[:])
    nc.sync.dma_start(out=out_d[:], in_=dcol[:])


@functools.lru_cache(maxsize=1)
def build_nc():
    from contextlib import ExitStack
    nc = bacc.Bacc(
        "TRN2", target_bir_lowering=False, debug=False, num_devices=CORES
    )
    with tile.TileContext(nc) as tc:
        with ExitStack() as ctx:
            _build_body.ctx = ctx
            _build_body(tc)
    nc.compile()
    return nc


# ======================================================================
# entry point
# ======================================================================

def kernel(**inputs) -> np.ndarray:
    in_maps, perms = make_in_maps(inputs)
    nc = build_nc()
    res = run_bass_kernel_spmd(nc, in_maps, core_ids=list(range(CORES)))
    return unshard(res.results, perms)


if __name__ == "__main__":
    import reference as ref_mod
    inputs = {k: np.asarray(v) for k, v in ref_mod.setup_inputs().items()}
    expected = np.asarray(ref_mod.reference(**inputs))
    actual = kernel(**inputs)
    rel = np.linalg.norm(actual - expected) / np.linalg.norm(expected)
    print("Relative error:", rel)

